# revision 7
# baseline (speedup 1.0000x reference)
"""Mixtral-style MoE layer on 8 Trainium2 NeuronCores (Bass/Tile).

Strategy: hybrid expert-parallel x tensor-parallel (EP4 x TP2).
Experts are paired (heaviest load with lightest) into 4 groups; group
g is served by cores 2g (F rows [0,2048)) and 2g+1 (F rows
[2048,4096)). Each core processes BOTH experts of its group against
its F-half: a fixed-size segment per expert (C1 for the heavy 4, C2
for the light 4 — identical across cores, as SPMD requires), so the
padded capacity per core is C1+C2 ~ 4160 instead of expert-parallel's
2*2112 = 4224. The two F-half partials of a group are summed on host.

Per-core compute (fp16 matmuls, fp32 accumulate):
    yT = w2_half @ (silu(w1_half @ xT) * (w3_half @ xT)) * s

Layouts (per core, seg in {0,1} selects the group's expert):
  xT   [KO, 128, C]            xT[k,p,c] = x_gathered[c, 128k+p]
  w1t  [2, 16, 128, KO, 128]   w1t[seg,f,p,k,m] = w1[e_seg][2048*hf+128f+m, 128k+p]
  w3t  same as w1t
  w2t  [2, HB, 128, 16, 128]   w2t[seg,h,p,f,m] = w2[e_seg][128h+m, 2048*hf+128f+p]
  sb   [128, C]                routing weight broadcast across partitions
  yT   [HB, 128, C]            partial output (fp32), summed per group on host
"""

import sys

sys.path.insert(0, "/opt/trn_rl_repo")

import numpy as np

import concourse.bass as bass  # noqa: F401  (bass must import before bacc)
from concourse import bacc
import concourse.mybir as mybir
import concourse.tile as tile
from concourse.bass_utils import run_bass_kernel_spmd

E = 8
TOP_K = 2
H = 2048
F = 4096
P = 128
KO = H // P      # 16  k-blocks for stage A contraction
FB2 = F // 2 // P  # 16  f-blocks per core (F/2)
HB = H // P      # 16  h-blocks

N_CORES = 8
F_HALF = F // 2  # 2048

F32 = mybir.dt.float32
F16 = mybir.dt.float16

_cache = {}


def _chunk_plan(C):
    """Chunks of <=1024 (64-aligned), split into PSUM-bank pieces of
    <=512. Chunks must stay >=~350 wide so weight-tile DMA hides under
    per-f compute; wider chunks mean fewer matmul instructions (~3ns
    fixed overhead each) and fewer PSUM accumulation groups."""
    n = max(1, -(-C // 1024))
    ch = 64 * (-(-C // (64 * n)))
    chunks = []
    left = C
    while left > 0:
        c = min(ch, left)
        chunks.append(c)
        left -= c
    plans = []
    off = 0
    for c in chunks:
        if c <= 512:
            subs = [(0, c)]
        else:
            half = min(512, 64 * (-(-c // 128)))
            subs = [(0, half), (half, c - half)]
        plans.append((off, c, subs))
        off += c
    return plans


def _build(C1, C2):
    """Build + schedule the Bass module for segment sizes (C1, C2)."""
    C = C1 + C2
    nc = bacc.Bacc(None, target_bir_lowering=False)

    xT = nc.dram_tensor("xT", [KO, P, C], F16, kind="ExternalInput")
    w1t = nc.dram_tensor("w1t", [2, FB2, P, KO, P], F16, kind="ExternalInput")
    w3t = nc.dram_tensor("w3t", [2, FB2, P, KO, P], F16, kind="ExternalInput")
    w2t = nc.dram_tensor("w2t", [2, HB, P, FB2, P], F16, kind="ExternalInput")
    sb = nc.dram_tensor("sb", [P, C], F32, kind="ExternalInput")
    yT = nc.dram_tensor("yT", [HB, P, C], F32, kind="ExternalOutput")

    # (segment, chunk) plan: segment 0 = cols [0, C1), segment 1 = rest
    plans = []
    for seg, (base, Cs) in enumerate(((0, C1), (C1, C2))):
        for off, CH, subs in _chunk_plan(Cs):
            plans.append((seg, base + off, CH, subs))
    CH0 = max(p[2] for p in plans)

    with tile.TileContext(nc) as tc:
        with (
            tc.tile_pool(name="xp", bufs=1) as xp,
            tc.tile_pool(name="hp", bufs=2) as hp,
            tc.tile_pool(name="wa", bufs=3) as wa,
            tc.tile_pool(name="wb", bufs=3) as wb,
            tc.tile_pool(name="tmp", bufs=4) as tmp,
            tc.tile_pool(name="yo", bufs=4) as yo,
            tc.tile_pool(name="cst", bufs=1) as cst,
            tc.tile_pool(name="ps", bufs=8, space="PSUM") as ps,
        ):
            s_tile = cst.tile([P, C], F32, tag="s")

            for ci, (seg, c0, CH, subs) in enumerate(plans):
                # first chunk: issue f=0 weight tiles before x so the
                # first matmul's stationary operand is in flight early
                w_pre = None
                if ci == 0:
                    # split the very first weight tile so the k=0 slice
                    # (all the first matmul needs) lands in ~1us
                    w1_0 = wa.tile([P, KO, P], F16, tag="w1")
                    nc.sync.dma_start(w1_0[:, 0:2], w1t[seg, 0, :, 0:2])
                    nc.scalar.dma_start(w1_0[:, 2:], w1t[seg, 0, :, 2:])
                    w3_0 = wa.tile([P, KO, P], F16, tag="w3")
                    nc.scalar.dma_start(w3_0[:], w3t[seg, 0])
                    w_pre = (w1_0, w3_0)

                x_tile = xp.tile([P, KO, CH0], F16, tag="x", name="x_tile")[:, :, :CH]
                # k-quarter DMAs alternating between the two DMA-capable
                # issue engines (sync + scalar): parallel issue chains
                # and distinct queues, and the first matmuls depend only
                # on the first quarter
                for qi, q in enumerate(range(0, KO, 4)):
                    eng = nc.sync if qi % 2 == 0 else nc.scalar
                    eng.dma_start(
                        x_tile[:, q : q + 4, :],
                        xT[q : q + 4, :, c0 : c0 + CH].rearrange("k p c -> p k c"),
                    )
                h_tile = hp.tile([P, FB2, CH0], F16, tag="h", name="h_tile")[:, :, :CH]

                # ---- stage A: h = silu(w1 @ x) * (w3 @ x) ----
                for f in range(FB2):
                    if f == 0 and w_pre is not None:
                        w1_tile, w3_tile = w_pre
                    else:
                        w1_tile = wa.tile([P, KO, P], F16, tag="w1")
                        nc.sync.dma_start(w1_tile[:], w1t[seg, f])
                        w3_tile = wa.tile([P, KO, P], F16, tag="w3")
                        nc.sync.dma_start(w3_tile[:], w3t[seg, f])
                    # piece-inner ordering: each stationary weight tile
                    # is reused for both PSUM pieces back-to-back,
                    # halving LDWEIGHTS pressure on the PE
                    pgs = [
                        ps.tile([P, 512], F32, tag="mm", name="mm")[:, :cw]
                        for _, cw in subs
                    ]
                    pus = [
                        ps.tile([P, 512], F32, tag="mm", name="mm")[:, :cw]
                        for _, cw in subs
                    ]
                    for k in range(KO):
                        for i, (cs, cw) in enumerate(subs):
                            nc.tensor.matmul(
                                pgs[i][:],
                                w1_tile[:, k, :],
                                x_tile[:, k, cs : cs + cw],
                                start=(k == 0),
                                stop=(k == KO - 1),
                            )
                    for k in range(KO):
                        for i, (cs, cw) in enumerate(subs):
                            nc.tensor.matmul(
                                pus[i][:],
                                w3_tile[:, k, :],
                                x_tile[:, k, cs : cs + cw],
                                start=(k == 0),
                                stop=(k == KO - 1),
                            )
                    for i, (cs, cw) in enumerate(subs):
                        sg = tmp.tile([P, 512], F32, tag="sg", name="sg")[:, :cw]
                        nc.scalar.activation(
                            sg[:], pgs[i][:], mybir.ActivationFunctionType.Silu
                        )
                        nc.vector.tensor_tensor(
                            h_tile[:, f, cs : cs + cw],
                            sg[:],
                            pus[i][:],
                            mybir.AluOpType.mult,
                        )

                if ci == 0:
                    # only read by stage B; keep it off the startup path
                    nc.sync.dma_start(s_tile[:], sb[:, :])

                # ---- stage B: yT = (w2 @ h) * s ----
                for hb in range(HB):
                    w2_tile = wb.tile([P, FB2, P], F16, tag="w2")
                    nc.sync.dma_start(w2_tile[:], w2t[seg, hb])
                    y_hb = yo.tile([P, CH0], F32, tag="y", name="y_hb")[:, :CH]
                    pys = [
                        ps.tile([P, 512], F32, tag="mm", name="mm")[:, :cw]
                        for _, cw in subs
                    ]
                    for f in range(FB2):
                        for i, (cs, cw) in enumerate(subs):
                            nc.tensor.matmul(
                                pys[i][:],
                                w2_tile[:, f, :],
                                h_tile[:, f, cs : cs + cw],
                                start=(f == 0),
                                stop=(f == FB2 - 1),
                            )
                    for i, (cs, cw) in enumerate(subs):
                        nc.vector.tensor_tensor(
                            y_hb[:, cs : cs + cw],
                            pys[i][:],
                            s_tile[:, c0 + cs : c0 + cs + cw],
                            mybir.AluOpType.mult,
                        )
                    nc.sync.dma_start(yT[hb, :, c0 : c0 + CH], y_hb[:])

    nc.compile()
    return nc


def kernel(hidden_states, gate_w, w1, w3, w2):
    x = np.ascontiguousarray(hidden_states, dtype=np.float32)
    gate_w = np.asarray(gate_w, dtype=np.float32)
    T = x.shape[0]

    # ---- host router (0.03% of FLOPs); exact jax ops to match the
    # reference's top-2 tie-breaking bit-for-bit ----
    import jax
    import jax.numpy as jnp

    router_logits = jnp.asarray(x) @ jnp.asarray(gate_w).T   # [T, E]
    probs = jax.nn.softmax(router_logits, axis=-1)
    topk_w, topk_ids = jax.lax.top_k(probs, TOP_K)
    topk_w = topk_w / jnp.sum(topk_w, axis=-1, keepdims=True)
    top2 = np.asarray(topk_ids)                              # [T, 2]
    tw = np.asarray(topk_w, dtype=np.float32)                # [T, 2]

    idx_e = []
    s_e = []
    for e in range(E):
        tok, slot = np.nonzero(top2 == e)
        idx_e.append(tok.astype(np.int64))
        s_e.append(tw[tok, slot].astype(np.float32))
    loads = np.array([len(ix) for ix in idx_e])

    # pair heaviest with lightest: 4 groups of (heavy, light)
    order = np.argsort(-loads, kind="stable")
    groups = [(int(order[i]), int(order[7 - i])) for i in range(4)]
    C1 = 64 * (-(-int(max(loads[a] for a, _ in groups)) // 64))
    C2 = 64 * (-(-int(max(loads[b] for _, b in groups)) // 64))
    C = C1 + C2

    if (C1, C2) not in _cache:
        _cache[(C1, C2)] = _build(C1, C2)
    nc = _cache[(C1, C2)]

    w1f = np.asarray(w1, np.float32)
    w3f = np.asarray(w3, np.float32)
    w2f = np.asarray(w2, np.float32)

    in_maps = []
    for g, (ea, eb) in enumerate(groups):
        xg = np.zeros((C, H), dtype=np.float32)
        sg_ = np.zeros((C,), dtype=np.float32)
        xg[: loads[ea]] = x[idx_e[ea]]
        sg_[: loads[ea]] = s_e[ea]
        xg[C1 : C1 + loads[eb]] = x[idx_e[eb]]
        sg_[C1 : C1 + loads[eb]] = s_e[eb]
        xT_h = np.ascontiguousarray(xg.T.reshape(KO, P, C).astype(np.float16))
        sb_h = np.ascontiguousarray(np.broadcast_to(sg_[None, :], (P, C)))
        for hf in range(2):
            fs = hf * F_HALF
            # stack segments: [2, F_HALF, H] with seg0=ea, seg1=eb
            w1seg = np.stack(
                [w1f[ea, fs : fs + F_HALF], w1f[eb, fs : fs + F_HALF]]
            )
            w3seg = np.stack(
                [w3f[ea, fs : fs + F_HALF], w3f[eb, fs : fs + F_HALF]]
            )
            w2seg = np.stack(
                [w2f[ea, :, fs : fs + F_HALF], w2f[eb, :, fs : fs + F_HALF]]
            )                                                # [2, H, F_HALF]
            in_maps.append(
                {
                    "xT": xT_h,
                    "w1t": np.ascontiguousarray(
                        w1seg.reshape(2, FB2, P, KO, P)
                        .transpose(0, 1, 4, 3, 2)
                        .astype(np.float16)
                    ),
                    "w3t": np.ascontiguousarray(
                        w3seg.reshape(2, FB2, P, KO, P)
                        .transpose(0, 1, 4, 3, 2)
                        .astype(np.float16)
                    ),
                    "w2t": np.ascontiguousarray(
                        w2seg.reshape(2, HB, P, FB2, P)
                        .transpose(0, 1, 4, 3, 2)
                        .astype(np.float16)
                    ),
                    "sb": sb_h,
                }
            )

    res = run_bass_kernel_spmd(nc, in_maps, core_ids=list(range(N_CORES)))

    out = np.zeros((T, H), dtype=np.float32)
    for g, (ea, eb) in enumerate(groups):
        y = (
            res.results[2 * g]["yT"].reshape(H, C)
            + res.results[2 * g + 1]["yT"].reshape(H, C)
        ).T                                                  # [C, H]
        # within one expert every token index is unique, so fancy-index
        # += is safe
        out[idx_e[ea]] += y[: loads[ea]]
        out[idx_e[eb]] += y[C1 : C1 + loads[eb]]
    return out


# revision 8
# speedup vs baseline: 1.0049x; 1.0049x over previous
"""Mixtral-style MoE layer on 8 Trainium2 NeuronCores (Bass/Tile).

Strategy: hybrid expert-parallel x tensor-parallel (EP4 x TP2).
Experts are paired (heaviest load with lightest) into 4 groups; group
g is served by cores 2g (F rows [0,2048)) and 2g+1 (F rows
[2048,4096)). Each core processes BOTH experts of its group against
its F-half: a fixed-size segment per expert (C1 for the heavy 4, C2
for the light 4 — identical across cores, as SPMD requires), so the
padded capacity per core is C1+C2 ~ 4160 instead of expert-parallel's
2*2112 = 4224. The two F-half partials of a group are summed on host.

Per-core compute (fp16 matmuls, fp32 accumulate):
    yT = w2_half @ (silu(w1_half @ xT) * (w3_half @ xT)) * s

Layouts (per core, seg in {0,1} selects the group's expert):
  xT   [KO, 128, C]            xT[k,p,c] = x_gathered[c, 128k+p]
  w1t  [2, 16, 128, KO, 128]   w1t[seg,f,p,k,m] = w1[e_seg][2048*hf+128f+m, 128k+p]
  w3t  same as w1t
  w2t  [2, HB, 128, 16, 128]   w2t[seg,h,p,f,m] = w2[e_seg][128h+m, 2048*hf+128f+p]
  sb   [128, C]                routing weight broadcast across partitions
  yT   [HB, 128, C]            partial output (fp32), summed per group on host
"""

import sys

sys.path.insert(0, "/opt/trn_rl_repo")

import numpy as np

import concourse.bass as bass  # noqa: F401  (bass must import before bacc)
from concourse import bacc
import concourse.mybir as mybir
import concourse.tile as tile
from concourse.bass_utils import run_bass_kernel_spmd

E = 8
TOP_K = 2
H = 2048
F = 4096
P = 128
KO = H // P      # 16  k-blocks for stage A contraction
FB2 = F // 2 // P  # 16  f-blocks per core (F/2)
HB = H // P      # 16  h-blocks

N_CORES = 8
F_HALF = F // 2  # 2048

F32 = mybir.dt.float32
F16 = mybir.dt.float16

_cache = {}


def _chunk_plan(C):
    """Chunks of <=704 (64-aligned), pieces <=384 (512-wide moving
    operands measured ~21% slower per column on hw; chunks must stay
    >=~350 wide so weight-tile DMA hides under per-f compute)."""
    n = max(1, -(-C // 704))
    ch = 64 * (-(-C // (64 * n)))
    chunks = []
    left = C
    while left > 0:
        c = min(ch, left)
        chunks.append(c)
        left -= c
    plans = []
    off = 0
    for c in chunks:
        if c <= 512:
            subs = [(0, c)]
        else:
            half = min(512, 64 * (-(-c // 128)))
            subs = [(0, half), (half, c - half)]
        plans.append((off, c, subs))
        off += c
    return plans


def _build(C1, C2):
    """Build + schedule the Bass module for segment sizes (C1, C2)."""
    C = C1 + C2
    nc = bacc.Bacc(None, target_bir_lowering=False)

    xT = nc.dram_tensor("xT", [KO, P, C], F16, kind="ExternalInput")
    w1t = nc.dram_tensor("w1t", [2, FB2, P, KO, P], F16, kind="ExternalInput")
    w3t = nc.dram_tensor("w3t", [2, FB2, P, KO, P], F16, kind="ExternalInput")
    w2t = nc.dram_tensor("w2t", [2, HB, P, FB2, P], F16, kind="ExternalInput")
    sb = nc.dram_tensor("sb", [P, C], F32, kind="ExternalInput")
    yT = nc.dram_tensor("yT", [HB, P, C], F32, kind="ExternalOutput")

    # (segment, chunk) plan: segment 0 = cols [0, C1), segment 1 = rest
    plans = []
    for seg, (base, Cs) in enumerate(((0, C1), (C1, C2))):
        for off, CH, subs in _chunk_plan(Cs):
            plans.append((seg, base + off, CH, subs))
    CH0 = max(p[2] for p in plans)

    with tile.TileContext(nc) as tc:
        with (
            tc.tile_pool(name="xp", bufs=1) as xp,
            tc.tile_pool(name="hp", bufs=2) as hp,
            tc.tile_pool(name="wa", bufs=3) as wa,
            tc.tile_pool(name="wb", bufs=3) as wb,
            tc.tile_pool(name="tmp", bufs=4) as tmp,
            tc.tile_pool(name="yo", bufs=4) as yo,
            tc.tile_pool(name="cst", bufs=1) as cst,
            tc.tile_pool(name="ps", bufs=8, space="PSUM") as ps,
        ):
            s_tile = cst.tile([P, C], F32, tag="s")

            for ci, (seg, c0, CH, subs) in enumerate(plans):
                # first chunk: issue f=0 weight tiles before x so the
                # first matmul's stationary operand is in flight early
                w_pre = None
                if ci == 0:
                    # split the very first weight tile so the k=0 slice
                    # (all the first matmul needs) lands in ~1us
                    w1_0 = wa.tile([P, KO, P], F16, tag="w1")
                    nc.sync.dma_start(w1_0[:, 0:2], w1t[seg, 0, :, 0:2])
                    nc.sync.dma_start(w1_0[:, 2:], w1t[seg, 0, :, 2:])
                    w3_0 = wa.tile([P, KO, P], F16, tag="w3")
                    nc.sync.dma_start(w3_0[:], w3t[seg, 0])
                    w_pre = (w1_0, w3_0)

                x_tile = xp.tile([P, KO, CH0], F16, tag="x", name="x_tile")[:, :, :CH]
                # k-quarter DMAs: short issue chain, spread across
                # queues, first matmuls depend only on the first quarter
                for q in range(0, KO, 4):
                    nc.sync.dma_start(
                        x_tile[:, q : q + 4, :],
                        xT[q : q + 4, :, c0 : c0 + CH].rearrange("k p c -> p k c"),
                    )
                h_tile = hp.tile([P, FB2, CH0], F16, tag="h", name="h_tile")[:, :, :CH]

                # ---- stage A: h = silu(w1 @ x) * (w3 @ x) ----
                for f in range(FB2):
                    if f == 0 and w_pre is not None:
                        w1_tile, w3_tile = w_pre
                    else:
                        w1_tile = wa.tile([P, KO, P], F16, tag="w1")
                        nc.sync.dma_start(w1_tile[:], w1t[seg, f])
                        w3_tile = wa.tile([P, KO, P], F16, tag="w3")
                        nc.sync.dma_start(w3_tile[:], w3t[seg, f])
                    # piece-inner ordering: each stationary weight tile
                    # is reused for both PSUM pieces back-to-back,
                    # halving LDWEIGHTS pressure on the PE
                    pgs = [
                        ps.tile([P, 512], F32, tag="mm", name="mm")[:, :cw]
                        for _, cw in subs
                    ]
                    pus = [
                        ps.tile([P, 512], F32, tag="mm", name="mm")[:, :cw]
                        for _, cw in subs
                    ]
                    for k in range(KO):
                        for i, (cs, cw) in enumerate(subs):
                            nc.tensor.matmul(
                                pgs[i][:],
                                w1_tile[:, k, :],
                                x_tile[:, k, cs : cs + cw],
                                start=(k == 0),
                                stop=(k == KO - 1),
                            )
                    for k in range(KO):
                        for i, (cs, cw) in enumerate(subs):
                            nc.tensor.matmul(
                                pus[i][:],
                                w3_tile[:, k, :],
                                x_tile[:, k, cs : cs + cw],
                                start=(k == 0),
                                stop=(k == KO - 1),
                            )
                    for i, (cs, cw) in enumerate(subs):
                        sg = tmp.tile([P, 512], F32, tag="sg", name="sg")[:, :cw]
                        nc.scalar.activation(
                            sg[:], pgs[i][:], mybir.ActivationFunctionType.Silu
                        )
                        nc.vector.tensor_tensor(
                            h_tile[:, f, cs : cs + cw],
                            sg[:],
                            pus[i][:],
                            mybir.AluOpType.mult,
                        )

                if ci == 0:
                    # only read by stage B; keep it off the startup path
                    nc.sync.dma_start(s_tile[:], sb[:, :])

                # ---- stage B: yT = (w2 @ h) * s ----
                for hb in range(HB):
                    w2_tile = wb.tile([P, FB2, P], F16, tag="w2")
                    nc.sync.dma_start(w2_tile[:], w2t[seg, hb])
                    y_hb = yo.tile([P, CH0], F32, tag="y", name="y_hb")[:, :CH]
                    pys = [
                        ps.tile([P, 512], F32, tag="mm", name="mm")[:, :cw]
                        for _, cw in subs
                    ]
                    for f in range(FB2):
                        for i, (cs, cw) in enumerate(subs):
                            nc.tensor.matmul(
                                pys[i][:],
                                w2_tile[:, f, :],
                                h_tile[:, f, cs : cs + cw],
                                start=(f == 0),
                                stop=(f == FB2 - 1),
                            )
                    for i, (cs, cw) in enumerate(subs):
                        nc.vector.tensor_tensor(
                            y_hb[:, cs : cs + cw],
                            pys[i][:],
                            s_tile[:, c0 + cs : c0 + cs + cw],
                            mybir.AluOpType.mult,
                        )
                    nc.sync.dma_start(yT[hb, :, c0 : c0 + CH], y_hb[:])

    nc.compile()
    return nc


def kernel(hidden_states, gate_w, w1, w3, w2):
    x = np.ascontiguousarray(hidden_states, dtype=np.float32)
    gate_w = np.asarray(gate_w, dtype=np.float32)
    T = x.shape[0]

    # ---- host router (0.03% of FLOPs); exact jax ops to match the
    # reference's top-2 tie-breaking bit-for-bit ----
    import jax
    import jax.numpy as jnp

    router_logits = jnp.asarray(x) @ jnp.asarray(gate_w).T   # [T, E]
    probs = jax.nn.softmax(router_logits, axis=-1)
    topk_w, topk_ids = jax.lax.top_k(probs, TOP_K)
    topk_w = topk_w / jnp.sum(topk_w, axis=-1, keepdims=True)
    top2 = np.asarray(topk_ids)                              # [T, 2]
    tw = np.asarray(topk_w, dtype=np.float32)                # [T, 2]

    idx_e = []
    s_e = []
    for e in range(E):
        tok, slot = np.nonzero(top2 == e)
        idx_e.append(tok.astype(np.int64))
        s_e.append(tw[tok, slot].astype(np.float32))
    loads = np.array([len(ix) for ix in idx_e])

    # pair heaviest with lightest: 4 groups of (heavy, light)
    order = np.argsort(-loads, kind="stable")
    groups = [(int(order[i]), int(order[7 - i])) for i in range(4)]
    C1 = 64 * (-(-int(max(loads[a] for a, _ in groups)) // 64))
    C2 = 64 * (-(-int(max(loads[b] for _, b in groups)) // 64))
    C = C1 + C2

    if (C1, C2) not in _cache:
        _cache[(C1, C2)] = _build(C1, C2)
    nc = _cache[(C1, C2)]

    w1f = np.asarray(w1, np.float32)
    w3f = np.asarray(w3, np.float32)
    w2f = np.asarray(w2, np.float32)

    in_maps = []
    for g, (ea, eb) in enumerate(groups):
        xg = np.zeros((C, H), dtype=np.float32)
        sg_ = np.zeros((C,), dtype=np.float32)
        xg[: loads[ea]] = x[idx_e[ea]]
        sg_[: loads[ea]] = s_e[ea]
        xg[C1 : C1 + loads[eb]] = x[idx_e[eb]]
        sg_[C1 : C1 + loads[eb]] = s_e[eb]
        xT_h = np.ascontiguousarray(xg.T.reshape(KO, P, C).astype(np.float16))
        sb_h = np.ascontiguousarray(np.broadcast_to(sg_[None, :], (P, C)))
        for hf in range(2):
            fs = hf * F_HALF
            # stack segments: [2, F_HALF, H] with seg0=ea, seg1=eb
            w1seg = np.stack(
                [w1f[ea, fs : fs + F_HALF], w1f[eb, fs : fs + F_HALF]]
            )
            w3seg = np.stack(
                [w3f[ea, fs : fs + F_HALF], w3f[eb, fs : fs + F_HALF]]
            )
            w2seg = np.stack(
                [w2f[ea, :, fs : fs + F_HALF], w2f[eb, :, fs : fs + F_HALF]]
            )                                                # [2, H, F_HALF]
            in_maps.append(
                {
                    "xT": xT_h,
                    "w1t": np.ascontiguousarray(
                        w1seg.reshape(2, FB2, P, KO, P)
                        .transpose(0, 1, 4, 3, 2)
                        .astype(np.float16)
                    ),
                    "w3t": np.ascontiguousarray(
                        w3seg.reshape(2, FB2, P, KO, P)
                        .transpose(0, 1, 4, 3, 2)
                        .astype(np.float16)
                    ),
                    "w2t": np.ascontiguousarray(
                        w2seg.reshape(2, HB, P, FB2, P)
                        .transpose(0, 1, 4, 3, 2)
                        .astype(np.float16)
                    ),
                    "sb": sb_h,
                }
            )

    res = run_bass_kernel_spmd(nc, in_maps, core_ids=list(range(N_CORES)))

    out = np.zeros((T, H), dtype=np.float32)
    for g, (ea, eb) in enumerate(groups):
        y = (
            res.results[2 * g]["yT"].reshape(H, C)
            + res.results[2 * g + 1]["yT"].reshape(H, C)
        ).T                                                  # [C, H]
        # within one expert every token index is unique, so fancy-index
        # += is safe
        out[idx_e[ea]] += y[: loads[ea]]
        out[idx_e[eb]] += y[C1 : C1 + loads[eb]]
    return out
